# revision 1
# baseline (speedup 1.0000x reference)
"""CRF loss (nn_CRFLoss_3753801417182) on 8 Trainium2 NeuronCores.

Strategy (hardcoded for B=128, T=4096, C=46, L=43, 8 cores):
  - Shard the TIME axis: core k handles t in [512k, 512k+512) for ALL 128
    sequences.  SBUF partition = sequence, free dim = local time.  Per-core
    masks/lengths become per-partition scalars, and each core's DMA is 128
    fully-contiguous 94KB rows.
  - Per step t the 2-state denominator DP is a 2x2 transfer matrix in
    probability space:
        M_t = [[b00, b01], [b10, b11]]
        b00 = sum_c w0[c] * p[c]   (w0 folds the 0->0 'O' arc and 0->0 labels)
        b10 = sum_c w1[c] * p[c]   (1->0 labels)
        b01 = c01 * p2, b11 = c11 * p2   (I- arcs), p = exp(log_probs[t])
    Steps past a sequence's length become the identity matrix.
  - alpha_final^T = [1,0]^T M_1 M_2 ... M_T, so each core computes the
    ordered product of its 512 matrices by a binary tree reduction
    (9 levels), renormalizing by the max entry every 2 levels and
    accumulating log-scales (this is exact logsumexp semantics up to fp
    rounding; entries tinier than max*1e-38 underflow to 0 harmlessly).
  - Numerator: no per-partition gather op exists on TRN2, so sum
    lp[b,t,label] as per-class bucket ops: scalar_tensor_tensor computes
    (masked_label == c) * lp[:, :, c] with a built-in per-partition
    accumulate over the resident lp shard.  Classes are paired
    (c, c+23) via a shifted second label plane: 23 DVE ops total.
  - The channel reduction runs as bf16 folds 46->23->12->6 (DVE 2x mode,
    with a zero pad column) so the 1x-rate tensor_reduce only reads 6.
  - Each core emits [128, 6]: 2x2 normalized partial matrix, log-scale,
    partial numerator sum.  The final cross-core combine (8 tiny 2x2
    matrix products per sequence + logs + sum) runs on host in float64.
"""

import numpy as np
import ml_dtypes

import concourse.bass as bass
import concourse.bacc as bacc
import concourse.tile as tile
import concourse.mybir as mybir

F32 = mybir.dt.float32
BF16 = mybir.dt.bfloat16
I32 = mybir.dt.int32

B = 128          # sequences (= SBUF partitions)
T = 4096
C = 46           # channels
L = 43
NCORES = 8
W = T // NCORES  # 512 steps per core
NCH = 8          # DMA/compute chunks per core
CH = W // NCH    # 64 steps per chunk
FCH = CH * C     # 2944 f32 per partition per chunk
CHALF = 23       # 46 = 23 + 23 channel fold

AL = mybir.AluOpType
AF = mybir.ActivationFunctionType
AX = mybir.AxisListType


def build_program():
    """One SPMD Bass program; all 8 cores run identical code on their shard."""
    nc = bacc.Bacc()

    lp_d = nc.declare_dram_parameter("lp", [B, W, C], F32, isOutput=False)
    lab_d = nc.declare_dram_parameter("lab", [B, W], I32, isOutput=False)
    thr_d = nc.declare_dram_parameter("thr", [B, 1], F32, isOutput=False)
    # slot-mixed bf16 weights: [CH, C] per set, step t uses row (t mod CH).
    # Each channel alternates the two bf16 neighbours of the exact f32
    # weight so the time-average matches it to ~gap/(2*CH) ~ 3e-5 rel;
    # this keeps the weight-mul in the DVE 2x bf16 mode without the
    # systematic den bias plain bf16 weights would give.
    wbs_d = nc.declare_dram_parameter("wbs", [CH * 2 * C], BF16, isOutput=False)
    cc_d = nc.declare_dram_parameter("cc", [2], F32, isOutput=False)
    out_d = nc.declare_dram_parameter("out", [B, 6], F32, isOutput=True)

    lp3 = lp_d[:].rearrange("b (n s) c -> b n (s c)", n=NCH)  # [B, NCH, FCH]

    def bcast_dram(handle, n_elem, offset=0):
        base = handle[:]
        return bass.AP(tensor=base.tensor, offset=offset,
                       ap=[[0, B], [1, n_elem]])

    with tile.TileContext(nc) as tc:
        with (
            tc.tile_pool(name="singles", bufs=1) as singles,
            tc.tile_pool(name="ptmp", bufs=2) as ptmp,
            tc.tile_pool(name="utmp", bufs=1) as utmp,
            tc.tile_pool(name="tree", bufs=1) as tree,
            tc.tile_pool(name="treetmp", bufs=1) as treetmp,
        ):
            # ---- engine clock warmup ----------------------------------------
            # The first Activation otherwise needs 3 sync waits (ACT table
            # load + DVE const-bias write + its DMA input) and walrus's
            # per-instruction sync-wait slots overflow.  A dummy exp absorbs
            # the table-load and const ticks up front.
            warm = singles.tile([B, 1], F32, tag="warm")
            nc.vector.memset(warm[:], 0.0)
            nc.scalar.activation(warm[:], warm[:], AF.Exp)

            # ---- small persistent tiles -------------------------------------
            # both weight sets interleaved: [CH, 2, C]
            wbt = singles.tile([B, CH * 2 * C], BF16, tag="wbt")
            nc.sync.dma_start(out=wbt[:], in_=bcast_dram(wbs_d, CH * 2 * C))
            thrt = singles.tile([B, 1], F32, tag="thrt")
            nc.sync.dma_start(out=thrt[:], in_=thr_d[:])
            c01t = singles.tile([B, 1], F32, tag="c01t")
            c11t = singles.tile([B, 1], F32, tag="c11t")
            nc.sync.dma_start(out=c01t[:], in_=bcast_dram(cc_d, 1, offset=0))
            nc.sync.dma_start(out=c11t[:], in_=bcast_dram(cc_d, 1, offset=1))
            # absorb the small-DMA ticks into the DVE clock one at a time so
            # no single downstream DVE instruction needs many sync waits
            smtouch = singles.tile([B, 4], F32, tag="smtouch")
            nc.vector.tensor_copy(smtouch[:, 0:1], wbt[:, 0:1])
            nc.vector.tensor_copy(smtouch[:, 2:3], thrt[:])
            nc.vector.tensor_copy(smtouch[:, 3:4], c01t[:])

            labt = singles.tile([B, W], I32, tag="labt")
            nc.sync.dma_start(out=labt[:], in_=lab_d[:])

            # valid-step mask m (f32 0/1) and its complement m1
            tiof = singles.tile([B, W], F32, tag="tiof")
            nc.gpsimd.iota(tiof[:], pattern=[[1, W]], base=0,
                           channel_multiplier=0,
                           allow_small_or_imprecise_dtypes=True)
            m = singles.tile([B, W], F32, tag="m")
            nc.vector.tensor_scalar(m[:], tiof[:], thrt[:], None, op0=AL.is_lt)
            m1 = singles.tile([B, W], F32, tag="m1")
            nc.vector.tensor_scalar(m1[:], m[:], -1.0, 1.0,
                                    op0=AL.mult, op1=AL.add)

            # ---- per-chunk heavy passes -------------------------------------
            # b-values interleaved: bint[:, t, 0] = b00, bint[:, t, 1] = b10
            bint = singles.tile([B, W, 2], F32, tag="bint")
            p2 = singles.tile([B, W], F32, tag="p2")

            # fold buffers: 46 -> 23(+zero pad col) -> 12 -> 6 -> reduce.
            # bf16 folds run at DVE 2x; the final 1x reduce only reads 6.
            vf = singles.tile([B, CH, 2, 24], BF16, tag="vf")
            nc.vector.memset(vf[:, :, :, 23:24], 0.0)
            v2f = singles.tile([B, CH, 2, 12], BF16, tag="v2f")
            v3f = singles.tile([B, CH, 2, 6], BF16, tag="v3f")

            # full log-prob shard stays resident for the numerator pass
            lpf = singles.tile([B, W * C], F32, tag="lpf")
            lpf3 = lpf[:].rearrange("b (t c) -> b t c", c=C)
            # per-chunk 1-column DVE "touches": the DVE vector clock absorbs
            # each chunk DMA tick early, so the later whole-tile readers
            # (45 class ops) don't each need 8 sync waits (walrus limit).
            touch = singles.tile([B, NCH], F32, tag="touch")

            for ch in range(NCH):
                sl = slice(ch * CH, (ch + 1) * CH)
                fsl = slice(ch * FCH, (ch + 1) * FCH)
                nc.sync.dma_start(out=lpf[:, fsl], in_=lp3[:, ch])
                nc.vector.tensor_copy(touch[:, ch:ch + 1],
                                      lpf[:, ch * FCH:ch * FCH + 1])

                # ACT absorbs the DVE tick here so the exp below needs only
                # (DMA, ACT-self) waits -- walrus allows 2 per instruction.
                nc.scalar.copy(warm[:], touch[:, ch:ch + 1])

                P = ptmp.tile([B, FCH], BF16, tag="P")
                nc.scalar.activation(P[:], lpf[:, fsl], AF.Exp)
                P3 = P[:].rearrange("b (s c) -> b s c", c=C)

                # both weight sets in one mul/fold/reduce (set axis = 2)
                Pb = P3.unsqueeze(2).broadcast_to((B, CH, 2, C))
                wb4 = wbt[:].rearrange("b (s k c) -> b s k c", k=2, c=C)
                u = utmp.tile([B, CH * 2 * C], BF16, tag="u")
                u4 = u[:].rearrange("b (s k c) -> b s k c", k=2, c=C)
                nc.vector.tensor_tensor(u4, Pb, wb4, op=AL.mult)
                nc.vector.tensor_tensor(vf[:, :, :, 0:CHALF],
                                        u4[:, :, :, 0:CHALF],
                                        u4[:, :, :, CHALF:C], op=AL.add)
                nc.vector.tensor_tensor(v2f[:], vf[:, :, :, 0:12],
                                        vf[:, :, :, 12:24], op=AL.add)
                nc.vector.tensor_tensor(v3f[:], v2f[:, :, :, 0:6],
                                        v2f[:, :, :, 6:12], op=AL.add)
                nc.vector.tensor_reduce(bint[:, sl, :], v3f[:], axis=AX.X,
                                        op=AL.add)

                # p2 = P[:, :, 2] (on ACT: keeps DVE free)
                nc.scalar.copy(p2[:, sl], P3[:, :, 2])

            # ---- numerator: per-class masked bucket sums --------------------
            # lblm = label * m  (masked labels become 0, matching no class);
            # classes are paired (c, c+23) into one op via a shifted second
            # label plane, halving the op count.
            lblf = singles.tile([B, W], F32, tag="lblf")
            nc.vector.tensor_copy(lblf[:], labt[:])
            lblm = singles.tile([B, W], F32, tag="lblm")
            nc.vector.tensor_tensor(lblm[:], lblf[:], m[:], op=AL.mult)
            lbl2 = singles.tile([B, W, 2], F32, tag="lbl2")
            nc.vector.tensor_copy(lbl2[:, :, 0], lblm[:])
            nc.vector.tensor_scalar(lbl2[:, :, 1], lblm[:], -23.0, None,
                                    op0=AL.add)
            junk2 = singles.tile([B, W, 2], F32, tag="junk2")
            acc = singles.tile([B, 24], F32, tag="acc")
            for c in range(1, CHALF):          # pairs (c, c+23), c = 1..22
                in1 = bass.AP(tensor=lpf.tensor, offset=c,
                              ap=[lpf[:].ap[0], [C, W], [CHALF, 2]])
                nc.vector.scalar_tensor_tensor(
                    junk2[:], lbl2[:], float(c), in1,
                    op0=AL.is_equal, op1=AL.mult,
                    accum_out=acc[:, c:c + 1])
            nc.vector.scalar_tensor_tensor(    # class 23 alone
                junk2[:, :, 0], lblm[:], 23.0, lpf3[:, :, CHALF],
                op0=AL.is_equal, op1=AL.mult,
                accum_out=acc[:, CHALF:CHALF + 1])
            numc = singles.tile([B, 1], F32, tag="numc")
            nc.vector.tensor_reduce(numc[:], acc[:, 1:24], axis=AX.X,
                                    op=AL.add)

            # ---- leaf transfer matrices [B, W, 4] ---------------------------
            EA = tree.tile([B, W, 4], F32, tag="EA")
            e = EA[:]
            b00v = bint[:, :, 0]
            b10v = bint[:, :, 1]
            # E0 = b00*m + m1 ; E1 = c01*p2*m ; E2 = b10*m ; E3 = c11*p2*m + m1
            nc.vector.tensor_tensor(e[:, :, 0], b00v, m[:], op=AL.mult)
            nc.vector.tensor_tensor(e[:, :, 0], e[:, :, 0], m1[:], op=AL.add)
            nc.vector.scalar_tensor_tensor(e[:, :, 1], p2[:], c01t[:], m[:],
                                           op0=AL.mult, op1=AL.mult)
            nc.vector.tensor_tensor(e[:, :, 2], b10v, m[:], op=AL.mult)
            nc.vector.scalar_tensor_tensor(e[:, :, 3], p2[:], c11t[:], m[:],
                                           op0=AL.mult, op1=AL.mult)
            nc.vector.tensor_tensor(e[:, :, 3], e[:, :, 3], m1[:], op=AL.add)

            # ---- binary tree of 2x2 products (9 levels) ---------------------
            RENORM = {2, 4, 6, 8}
            cur = EA
            curw = W
            ls = None          # running log-scale tile [B, w]
            for lvl in range(1, 10):
                w = curw // 2
                nxt = tree.tile([B, w, 4], F32, tag=f"E{lvl}")
                x4 = cur[:, 0:curw].rearrange("b (w two) e -> b w two e", two=2)
                Lm = x4[:, :, 0]      # [B, w, 4] left matrices
                Rm = x4[:, :, 1]      # right matrices
                # batched 2x2 product: one mul covers both output rows.
                # L4[b,w,r,k] = L[2r] (k-dup) / L[2r+1]; R4[b,w,r,k] = Rrow
                L4 = Lm.rearrange("b w (r two) -> b w r two", two=2)
                Lc0 = L4[:, :, :, 0:1].broadcast_to((B, w, 2, 2))  # l00,l10
                Lc1 = L4[:, :, :, 1:2].broadcast_to((B, w, 2, 2))  # l01,l11
                Rr0 = Rm[:, :, 0:2].unsqueeze(2).broadcast_to((B, w, 2, 2))
                Rr1 = Rm[:, :, 2:4].unsqueeze(2).broadcast_to((B, w, 2, 2))
                o = nxt[:]
                o4 = o.rearrange("b w (r two) -> b w r two", two=2)
                u = treetmp.tile([B, w, 4], F32, tag="tu")
                v = treetmp.tile([B, w, 4], F32, tag="tv")
                u4 = u[:].rearrange("b w (r two) -> b w r two", two=2)
                v4 = v[:].rearrange("b w (r two) -> b w r two", two=2)
                nc.vector.tensor_tensor(u4, Lc0, Rr0, op=AL.mult)
                nc.vector.tensor_tensor(v4, Lc1, Rr1, op=AL.mult)
                nc.vector.tensor_tensor(o4, u4, v4, op=AL.add)

                # fold the log-scale pair-sum alongside
                if ls is not None:
                    ls2 = ls[:].rearrange("b (w two) -> b w two", two=2)
                    lsn = tree.tile([B, w], F32, tag=f"ls{lvl}")
                    nc.vector.tensor_tensor(lsn[:], ls2[:, :, 0], ls2[:, :, 1],
                                            op=AL.add)
                    ls = lsn

                if lvl in RENORM:
                    mx = treetmp.tile([B, w], F32, tag="mx")
                    nc.vector.tensor_reduce(mx[:], o, axis=AX.X, op=AL.max)
                    rc = treetmp.tile([B, w], F32, tag="rc")
                    nc.vector.reciprocal(rc[:], mx[:])
                    rcb = rc[:].unsqueeze(2).broadcast_to((B, w, 4))
                    nc.vector.tensor_tensor(o, o, rcb, op=AL.mult)
                    lg = tree.tile([B, w], F32, tag=f"lg{lvl}")
                    nc.scalar.activation(lg[:], mx[:], AF.Ln)
                    if ls is None:
                        ls = lg
                    else:
                        nc.vector.tensor_tensor(ls[:], ls[:], lg[:], op=AL.add)

                cur = nxt
                curw = w

            # ---- pack [B, 6] = (E00,E01,E10,E11, ls, num) and store ---------
            o6 = singles.tile([B, 6], F32, tag="o6")
            nc.vector.tensor_copy(o6[:, 0:4],
                                  cur[:].rearrange("b one e -> b (one e)"))
            nc.vector.tensor_copy(o6[:, 4:5], ls[:])
            nc.vector.tensor_copy(o6[:, 5:6], numc[:])
            nc.sync.dma_start(out=out_d[:], in_=o6[:])

    if not nc.is_finalized():
        nc.finalize()
    return nc


def _log_softmax_np(x):
    x = np.asarray(x, np.float64)
    m = x.max()
    e = np.exp(x - m)
    return x - m - np.log(e.sum())


def _mk_slots(w):
    """[CH, C] bf16 rows: per channel a Bresenham mix of the two bf16
    neighbours of w[c] whose time-average matches w[c] to ~gap/(2*CH)."""
    slots = np.zeros((CH, w.size), np.float64)
    for c, wc in enumerate(np.asarray(w, np.float64)):
        wd = np.float64(ml_dtypes.bfloat16(wc))
        if wd > wc:
            wu = wd
            wd = np.float64(np.nextafter(ml_dtypes.bfloat16(wc),
                                         ml_dtypes.bfloat16(-np.inf)))
        else:
            wu = np.float64(np.nextafter(ml_dtypes.bfloat16(wc),
                                         ml_dtypes.bfloat16(np.inf)))
        gap = wu - wd
        lam = 0.0 if gap <= 0 else (wc - wd) / gap
        nup = int(round(lam * CH))
        j = np.arange(CH)
        ups = np.floor((j + 1) * nup / CH) > np.floor(j * nup / CH)
        slots[:, c] = np.where(ups, wu, wd)
    return slots.astype(ml_dtypes.bfloat16)


def make_in_maps(log_probs, den_params, input_lens, labels):
    g0 = _log_softmax_np(den_params[:L + 3])
    g1 = _log_softmax_np(den_params[L + 3:])
    w0 = np.zeros(C, np.float64)
    w0[1] = np.exp(g0[0])              # 0->0 on 'O' reads lp[:,1]
    w0[3:] = np.exp(g0[1:L + 1])       # 0->0 labels
    w1 = np.zeros(C, np.float64)
    w1[3:] = np.exp(g1[1:])            # 1->0 labels
    c01 = np.exp(g0[L + 1])            # 0->1 on 'I-' (emits lp[:,2])
    c11 = np.exp(g1[0])                # 1->1 on 'I-'
    s_fin = g0[L + 2]

    wbs = np.stack([_mk_slots(w0), _mk_slots(w1)], axis=1).reshape(-1)
    in_maps = []
    for k in range(NCORES):
        sl = slice(W * k, W * (k + 1))
        in_maps.append({
            "lp": np.ascontiguousarray(log_probs[:, sl, :], np.float32),
            "lab": np.ascontiguousarray(labels[:, sl]).astype(np.int32),
            "thr": np.clip(input_lens.astype(np.int64) - W * k, 0, W)
                     .astype(np.float32)[:, None],
            "wbs": wbs,
            "cc": np.array([c01, c11], np.float32),
        })
    return in_maps, s_fin


def combine_partials(parts, s_fin):
    """parts: list of 8 arrays [B, 6].  Host-side f64 final combine."""
    num = np.zeros(B, np.float64)
    ls = np.zeros(B, np.float64)
    a = np.zeros((B, 2), np.float64)
    a[:, 0] = 1.0
    for k in range(NCORES):
        p = np.asarray(parts[k], np.float64)
        Mk = p[:, 0:4].reshape(B, 2, 2)
        a = np.einsum("bi,bij->bj", a, Mk)
        s = np.abs(a).max(axis=1)
        a /= s[:, None]
        ls += np.log(s) + p[:, 4]
        num += p[:, 5]
    den = np.log(a[:, 0]) + ls + s_fin
    return np.float32((num - den).sum())


_NC_CACHE = None


def kernel(log_probs, den_params, input_lens, labels):
    global _NC_CACHE
    from concourse.bass_utils import run_bass_kernel_spmd

    log_probs = np.asarray(log_probs)
    den_params = np.asarray(den_params)
    input_lens = np.asarray(input_lens)
    labels = np.asarray(labels)

    if _NC_CACHE is None:
        _NC_CACHE = build_program()
    nc = _NC_CACHE

    in_maps, s_fin = make_in_maps(log_probs, den_params, input_lens, labels)
    res = run_bass_kernel_spmd(nc, in_maps, list(range(NCORES))).results
    parts = [res[k]["out"] for k in range(NCORES)]
    return combine_partials(parts, s_fin)

